# revision 11
# baseline (speedup 1.0000x reference)
"""Trainium2 Bass kernel for nn_CAS_Decoder (3-stage attention+GRU decoder).

Strategy: data-parallel over batch B=128 across 8 NeuronCores (16 rows each).
All matmul operands are pre-transposed/bf16-converted on the host so every
DMA is a contiguous row-block load; accumulation stays fp32 on-chip.

Layout conventions per core (Bl=16 local batch, L=96, H=1024, R=Bl*L=1536):
  - "xT" tensors: [features, batch] stored in SBUF as [128, 16*n_chunks]
    (feature chunk k lives at columns 16k:16k+16).
  - encoder stream j: e_nat [R, H] (row r = l*16+b) and e_T [H, R], both bf16.
  - energy computed transposed: psum[ho_chunk, r] = W2T.T @ e_T chunks.
  - scores via PE dot with v; softmax on [16, 96]; context via PE with the
    per-tile softmax-weight matrix A (built with a host mask and DVE).
"""
import numpy as np
import ml_dtypes

H = 1024
B = 128
L = 96
C = 3
A_ = 32
S = 64
DEG = 6
NC = 8
BL = B // NC          # 16 local batch rows
R = BL * L            # 1536
KH = H // 128         # 8 chunks of H
RT = R // 128         # 12 row tiles
NRG = R // 512        # 3 r-groups of 512
GK = 57               # gin chunks: 8 x + 48 ctx + 1 kb/bias
GM = 24               # 3072/128 gate chunks

bf16 = ml_dtypes.bfloat16

_CACHE = {}


def _build():
    import concourse.bacc as bacc
    import concourse.mybir as mybir
    from concourse.tile import TileContext

    f32 = mybir.dt.float32
    b16 = mybir.dt.bfloat16
    TANH = mybir.ActivationFunctionType.Tanh
    SIG = mybir.ActivationFunctionType.Sigmoid
    EXP = mybir.ActivationFunctionType.Exp
    CPY = mybir.ActivationFunctionType.Copy

    nc = bacc.Bacc("TRN2", target_bir_lowering=False, debug=False, num_devices=NC)

    def P(name, shape, dt=b16):
        return nc.declare_dram_parameter(name, list(shape), dt, isOutput=False)

    eT = [P(f"eT{j}", (H, R)) for j in range(6)]
    en = [P(f"en{j}", (R, H)) for j in range(6)]
    W1T = P("W1T", (3, 6, 1152, H))
    W2T = P("W2T", (3, 6, H, H))
    vT = P("vT", (3, 6, H))
    WihT = P("WihT", (3, GK * 128, 3 * H))
    WhhT = P("WhhT", (3, 1152, 3 * H))
    WoT = P("WoT", (3, GK * 128, 64))
    WinT = [P(f"WinT{i}", (n * 128, H)) for i, n in enumerate((9, 2, 4))]
    cat0T = P("cat0T", (128, 144))
    base1T_in = P("base1T_in", (128, 16))
    base2T_in = P("base2T_in", (128, 48))
    kbpadT = P("kbpadT", (128, 16))
    h0T = P("h0T", (128, 128), f32)
    ident_f = P("ident_f", (128, 128), f32)
    ident_b = P("ident_b", (128, 128))
    ones16 = P("ones16", (128, 16))
    mask16 = P("mask16", (128, 16))

    out_cont = nc.declare_dram_parameter("out_cont", [BL, C], f32, isOutput=True)
    out_act = nc.declare_dram_parameter("out_act", [BL, A_], f32, isOutput=True)
    out_slot = nc.declare_dram_parameter("out_slot", [BL, S], f32, isOutput=True)
    out_h = nc.declare_dram_parameter("out_h", [BL, H], f32, isOutput=True)

    from contextlib import ExitStack

    with TileContext(nc) as tc, ExitStack() as ctx:
        cst = ctx.enter_context(tc.tile_pool(name="cst", bufs=1))
        hpool = ctx.enter_context(tc.tile_pool(name="hpool", bufs=2))
        wpool = ctx.enter_context(tc.tile_pool(name="wpool", bufs=1))
        epool = ctx.enter_context(tc.tile_pool(name="epool", bufs=8))
        w2pool = ctx.enter_context(tc.tile_pool(name="w2pool", bufs=8))
        w1pool = ctx.enter_context(tc.tile_pool(name="w1pool", bufs=9))
        enpool = ctx.enter_context(tc.tile_pool(name="enpool", bufs=12))
        vpool = ctx.enter_context(tc.tile_pool(name="vpool", bufs=2))
        thpool = ctx.enter_context(tc.tile_pool(name="thpool", bufs=3))
        tmpool = ctx.enter_context(tc.tile_pool(name="tmpool", bufs=3))
        qpool = ctx.enter_context(tc.tile_pool(name="qpool", bufs=2))
        smpool = ctx.enter_context(tc.tile_pool(name="smpool", bufs=2))
        gpool = ctx.enter_context(tc.tile_pool(name="gpool", bufs=2))
        wgpool = ctx.enter_context(tc.tile_pool(name="wgpool", bufs=5))
        wopool = ctx.enter_context(tc.tile_pool(name="wopool", bufs=1))
        gsb = ctx.enter_context(tc.tile_pool(name="gsb", bufs=1))
        drp = ctx.enter_context(tc.tile_pool(name="drp", bufs=2, space="DRAM"))

        # ---- constants ----
        idf = cst.tile([128, 128], f32, tag="idf")
        nc.sync.dma_start(idf[:], ident_f[:])
        idb = cst.tile([128, 128], b16, tag="idb")
        nc.sync.dma_start(idb[:], ident_b[:])
        one_t = cst.tile([128, 16], b16, tag="one_t")
        nc.sync.dma_start(one_t[:], ones16[:])
        msk = cst.tile([128, 16], b16, tag="msk")
        nc.sync.dma_start(msk[:], mask16[:])
        kbp = cst.tile([128, 16], b16, tag="kbp")
        nc.sync.dma_start(kbp[:], kbpadT[:])
        b1t = cst.tile([128, 16], b16, tag="b1t")
        nc.sync.dma_start(b1t[:], base1T_in[:])
        b2t = cst.tile([128, 48], b16, tag="b2t")
        nc.sync.dma_start(b2t[:], base2T_in[:])
        c0t = cst.tile([128, 144], b16, tag="c0t")
        nc.sync.dma_start(c0t[:], cat0T[:])
        win_sb = []
        for i, nk in enumerate((9, 2, 4)):
            w = wpool.tile([128, nk * 1024], b16, tag=f"win{i}")
            nc.sync.dma_start(
                w[:].rearrange("p (k n) -> p k n", n=1024),
                WinT[i][:].rearrange("(k p) n -> p k n", p=128))
            win_sb.append(w)

        hT_f = hpool.tile([128, 128], f32, tag="hT_f")
        nc.sync.dma_start(hT_f[:], h0T[:])
        hT_b = hpool.tile([128, 128], b16, tag="hT_b")
        nc.vector.tensor_copy(hT_b[:], hT_f[:])

        mm = nc.tensor.matmul

        for i in range(3):
            # ================= input projection =================
            with tc.tile_pool(name="px", bufs=1, space="PSUM") as pxp:
                psum_x = pxp.tile([128, 128], f32)
                if i == 0:
                    rhs_chunks = [c0t[:, 16 * k:16 * k + 16] for k in range(9)]
                elif i == 1:
                    rhs_chunks = [b1t[:], one_t[:]]
                else:
                    rhs_chunks = [b2t[:, 0:16], b2t[:, 16:32], b2t[:, 32:48], one_t[:]]
                nck = len(rhs_chunks)
                gin = gpool.tile([128, 16 * 56], b16, tag="gin")
                for m in range(8):
                    for k in range(nck):
                        mm(psum_x[:, 16 * m:16 * m + 16],
                           win_sb[i][:, 1024 * k + 128 * m:1024 * k + 128 * m + 128],
                           rhs_chunks[k],
                           start=(m == 0 and k == 0), stop=(m == 7 and k == nck - 1))
                nc.scalar.activation(gin[:, 0:128], psum_x[:], TANH)

            # ================= attention (6 streams) =================
            with tc.tile_pool(name="pe_", bufs=2, space="PSUM") as pep, \
                 tc.tile_pool(name="ps_", bufs=2, space="PSUM") as psp, \
                 tc.tile_pool(name="pq_", bufs=2, space="PSUM") as pqp, \
                 tc.tile_pool(name="pc_", bufs=2, space="PSUM") as pcp:
                for j in range(6):
                    e_sl = []
                    for k in range(KH):
                        t = epool.tile([128, R], b16, tag="eT")
                        nc.sync.dma_start(t[:], eT[j][128 * k:128 * (k + 1), :])
                        e_sl.append(t)
                    w2_sl = []
                    for k in range(KH):
                        t = w2pool.tile([128, H], b16, tag="w2")
                        nc.sync.dma_start(t[:], W2T[i, j, 128 * k:128 * (k + 1), :])
                        w2_sl.append(t)
                    w1_sl = []
                    for k in range(9):
                        t = w1pool.tile([128, H], b16, tag="w1")
                        nc.sync.dma_start(t[:], W1T[i, j, 128 * k:128 * (k + 1), :])
                        w1_sl.append(t)
                    en_sl = []
                    for k in range(RT):
                        t = enpool.tile([128, H], b16, tag="en")
                        nc.sync.dma_start(t[:], en[j][128 * k:128 * (k + 1), :])
                        en_sl.append(t)
                    v_sb = vpool.tile([128, KH], b16, tag="v")
                    nc.sync.dma_start(v_sb[:], vT[i, j].rearrange("(k p) -> p k", p=128))

                    # qp.T = W1 @ x.T + b  -> [128, 8*16] fp32
                    psum_qp = pqp.tile([128, 128], f32)
                    for m in range(8):
                        for k in range(9):
                            rhs = gin[:, 16 * k:16 * k + 16] if k < 8 else one_t[:]
                            mm(psum_qp[:, 16 * m:16 * m + 16],
                               w1_sl[k][:, 128 * m:128 * m + 128], rhs,
                               start=(m == 0 and k == 0), stop=(m == 7 and k == 8))
                    qpT = qpool.tile([128, 128], f32, tag="qpT")
                    nc.vector.tensor_copy(qpT[:], psum_qp[:])

                    scf = smpool.tile([1, R], f32, tag="scf")
                    for rg in range(NRG):
                        psum_s = psp.tile([1, 512], f32)
                        for kho in range(KH):
                            psum_e = pep.tile([128, 512], f32)
                            for khi in range(KH):
                                mm(psum_e[:],
                                   w2_sl[khi][:, 128 * kho:128 * kho + 128],
                                   e_sl[khi][:, 512 * rg:512 * rg + 512],
                                   start=(khi == 0), stop=(khi == KH - 1))
                            tmp = tmpool.tile([128, 512], f32, tag="tmp")
                            qslice = qpT[:, 16 * kho:16 * kho + 16]
                            qb = qslice.unsqueeze(1).broadcast_to((128, 32, 16))
                            nc.vector.tensor_add(
                                tmp[:].rearrange("p (a b) -> p a b", b=16),
                                psum_e[:].rearrange("p (a b) -> p a b", b=16),
                                qb)
                            th = thpool.tile([128, 512], b16, tag="th")
                            nc.scalar.activation(th[:], tmp[:], TANH)
                            mm(psum_s[:], v_sb[:, kho:kho + 1], th[:],
                               start=(kho == 0), stop=(kho == KH - 1))
                        nc.scalar.activation(
                            scf[:, 512 * rg:512 * rg + 512], psum_s[:], CPY)

                    # softmax over L for each b
                    scr1 = drp.tile([R], f32, tag="scr1")
                    nc.gpsimd.dma_start(scr1[:], scf[:])
                    sbl = smpool.tile([16, L], f32, tag="sbl")
                    nc.gpsimd.dma_start(
                        sbl[:], scr1[:].rearrange("(l b) -> b l", b=16))
                    mx = smpool.tile([16, 1], f32, tag="mx")
                    nc.vector.reduce_max(mx[:], sbl[:], axis=mybir.AxisListType.X)
                    ngm = smpool.tile([16, 1], f32, tag="ngm")
                    nc.vector.tensor_scalar_mul(ngm[:], mx[:], -1.0)
                    es = smpool.tile([16, L], f32, tag="es")
                    ssum = smpool.tile([16, 1], f32, tag="ssum")
                    nc.scalar.activation(es[:], sbl[:], EXP, bias=ngm[:],
                                         accum_out=ssum[:])
                    rs = smpool.tile([16, 1], f32, tag="rs")
                    nc.vector.reciprocal(rs[:], ssum[:])
                    abl = smpool.tile([16, L], f32, tag="abl")
                    nc.vector.tensor_scalar_mul(abl[:], es[:], rs[:])
                    scr2 = drp.tile([16, L], f32, tag="scr2")
                    nc.gpsimd.dma_start(scr2[:], abl[:])
                    acol = smpool.tile([128, RT], f32, tag="acol")
                    nc.gpsimd.dma_start(
                        acol[:],
                        scr2[:].rearrange("b (t l0) -> l0 b t", l0=8))
                    Aall = smpool.tile([128, 16 * RT], b16, tag="Aall")
                    for t in range(RT):
                        nc.vector.tensor_scalar_mul(
                            Aall[:, 16 * t:16 * t + 16], msk[:], acol[:, t:t + 1])

                    # ctx.T accumulation -> gin chunk columns
                    psum_c = pcp.tile([128, 128], f32)
                    for m in range(8):
                        for t in range(RT):
                            mm(psum_c[:, 16 * m:16 * m + 16],
                               en_sl[t][:, 128 * m:128 * m + 128],
                               Aall[:, 16 * t:16 * t + 16],
                               start=(m == 0 and t == 0),
                               stop=(m == 7 and t == RT - 1))
                    nc.vector.tensor_copy(
                        gin[:, 128 * (1 + j):128 * (2 + j)], psum_c[:])

            # ================= GRU =================
            with tc.tile_pool(name="pg_", bufs=1, space="PSUM") as pgp, \
                 tc.tile_pool(name="pi_", bufs=1, space="PSUM") as pip_, \
                 tc.tile_pool(name="ph_", bufs=1, space="PSUM") as php:
                psum_g = pgp.tile([128, 256], f32)
                psum_in = pip_.tile([128, 128], f32)
                psum_hn = php.tile([128, 128], f32)
                for k in range(GK):
                    wsl = wgpool.tile([128, 3 * H], b16, tag="wg")
                    nc.sync.dma_start(wsl[:], WihT[i, 128 * k:128 * (k + 1), :])
                    rhs = gin[:, 16 * k:16 * k + 16] if k < 56 else kbp[:]
                    for m in range(GM):
                        lhs = wsl[:, 128 * m:128 * m + 128]
                        if m < 16:
                            mm(psum_g[:, 16 * m:16 * m + 16], lhs, rhs,
                               start=(k == 0 and m == 0), stop=False)
                        else:
                            mm(psum_in[:, 16 * (m - 16):16 * (m - 16) + 16], lhs, rhs,
                               start=(k == 0 and m == 16),
                               stop=(k == GK - 1 and m == GM - 1))
                for k in range(9):
                    wsl = wgpool.tile([128, 3 * H], b16, tag="wg")
                    nc.sync.dma_start(wsl[:], WhhT[i, 128 * k:128 * (k + 1), :])
                    rhs = hT_b[:, 16 * k:16 * k + 16] if k < 8 else one_t[:]
                    for m in range(GM):
                        lhs = wsl[:, 128 * m:128 * m + 128]
                        if m < 16:
                            mm(psum_g[:, 16 * m:16 * m + 16], lhs, rhs,
                               start=False, stop=(k == 8 and m == 15))
                        else:
                            mm(psum_hn[:, 16 * (m - 16):16 * (m - 16) + 16], lhs, rhs,
                               start=(k == 0 and m == 16),
                               stop=(k == 8 and m == GM - 1))
                r_sb = gsb.tile([128, 128], f32, tag="r_sb")
                nc.scalar.activation(r_sb[:], psum_g[:, 0:128], SIG)
                z_sb = gsb.tile([128, 128], f32, tag="z_sb")
                nc.scalar.activation(z_sb[:], psum_g[:, 128:256], SIG)
                t1 = gsb.tile([128, 128], f32, tag="t1")
                nc.vector.tensor_mul(t1[:], r_sb[:], psum_hn[:])
                t2 = gsb.tile([128, 128], f32, tag="t2")
                nc.vector.tensor_add(t2[:], t1[:], psum_in[:])
                n_sb = gsb.tile([128, 128], f32, tag="n_sb")
                nc.scalar.activation(n_sb[:], t2[:], TANH)
                dd = gsb.tile([128, 128], f32, tag="dd")
                nc.vector.tensor_sub(dd[:], hT_f[:], n_sb[:])
                t3 = gsb.tile([128, 128], f32, tag="t3")
                nc.vector.tensor_mul(t3[:], z_sb[:], dd[:])
                hp_f = hpool.tile([128, 128], f32, tag="hT_f")
                nc.vector.tensor_add(hp_f[:], n_sb[:], t3[:])
                hp_b = hpool.tile([128, 128], b16, tag="hT_b")
                nc.vector.tensor_copy(hp_b[:], hp_f[:])

            # ================= output projection =================
            with tc.tile_pool(name="po_", bufs=1, space="PSUM") as pop, \
                 tc.tile_pool(name="pt_", bufs=1, space="PSUM") as ptp, \
                 tc.tile_pool(name="pb_", bufs=1, space="PSUM") as pbp:
                wo_sb = wopool.tile([128, GK * 64], b16, tag="wo")
                nc.sync.dma_start(
                    wo_sb[:].rearrange("p (k c) -> p k c", c=64),
                    WoT[i].rearrange("(k p) c -> p k c", p=128))
                psum_o = pop.tile([64, 16], f32)
                for k in range(GK):
                    if k < 8:
                        rhs = hp_b[:, 16 * k:16 * k + 16]
                    elif k < 56:
                        rhs = gin[:, 16 * k:16 * k + 16]
                    else:
                        rhs = one_t[:]
                    mm(psum_o[:], wo_sb[:, 64 * k:64 * k + 64], rhs,
                       start=(k == 0), stop=(k == GK - 1))
                oT_sb = gsb.tile([64, 16], f32, tag="oT_sb")
                nc.vector.tensor_copy(oT_sb[:], psum_o[:])
                psum_t = ptp.tile([16, 64], f32)
                nc.tensor.transpose(psum_t[:], oT_sb[:], idf[0:64, 0:64])
                Cd = (C, A_, S)[i]
                onat = gsb.tile([16, 64], f32, tag="onat")
                if i < 2:
                    mx2 = smpool.tile([16, 1], f32, tag="mx2")
                    nc.vector.reduce_max(mx2[:], psum_t[:, 0:Cd],
                                         axis=mybir.AxisListType.X)
                    ngm2 = smpool.tile([16, 1], f32, tag="ngm2")
                    nc.vector.tensor_scalar_mul(ngm2[:], mx2[:], -1.0)
                    es2 = smpool.tile([16, 64], f32, tag="es2")
                    ssum2 = smpool.tile([16, 1], f32, tag="ssum2")
                    nc.scalar.activation(es2[:, 0:Cd], psum_t[:, 0:Cd], EXP,
                                         bias=ngm2[:], accum_out=ssum2[:])
                    rs2 = smpool.tile([16, 1], f32, tag="rs2")
                    nc.vector.reciprocal(rs2[:], ssum2[:])
                    nc.vector.tensor_scalar_mul(onat[:, 0:Cd], es2[:, 0:Cd], rs2[:])
                else:
                    nc.scalar.activation(onat[:, 0:Cd], psum_t[:, 0:Cd], SIG)
                tgt = (out_cont, out_act, out_slot)[i]
                nc.gpsimd.dma_start(tgt[:], onat[:, 0:Cd])
                if i < 2:
                    ob = gsb.tile([16, 64], b16, tag="ob")
                    nc.vector.tensor_copy(ob[:, 0:Cd], onat[:, 0:Cd])
                    psum_b = pbp.tile([64, 16], b16)
                    nc.tensor.transpose(psum_b[0:Cd, :], ob[:, 0:Cd],
                                        idb[0:16, 0:16])
                    if i == 0:
                        nc.vector.tensor_copy(b1t[0:3, :], psum_b[0:3, :])
                        nc.vector.tensor_copy(b2t[0:3, 0:16], psum_b[0:3, :])
                    else:
                        nc.vector.tensor_copy(b2t[0:32, 16:32], psum_b[0:32, :])

            hT_f = hp_f
            hT_b = hp_b

        # ---- final hidden state out ----
        with tc.tile_pool(name="pht", bufs=2, space="PSUM") as phtp:
            h_nat = gsb.tile([16, H], f32, tag="h_nat")
            for m in range(8):
                psum_ht = phtp.tile([16, 128], f32)
                nc.tensor.transpose(psum_ht[:], hT_f[:, 16 * m:16 * m + 16], idf[:])
                nc.scalar.activation(h_nat[:, 128 * m:128 * m + 128], psum_ht[:], CPY)
            nc.gpsimd.dma_start(out_h[:], h_nat[:])

    import os
    if not os.environ.get("KERNEL_TRACE_ONLY"):
        nc.compile()
    return nc


def _to_bf16(x):
    return np.ascontiguousarray(np.asarray(x, np.float32)).astype(bf16)


def _fold(xT):
    """[K, 16] -> SBUF layout [128, 16*ceil(K/128)] (chunk k at cols 16k)."""
    K = xT.shape[0]
    nk = (K + 127) // 128
    out = np.zeros((128, 16 * nk), xT.dtype)
    for k in range(nk):
        rows = xT[128 * k:128 * (k + 1)]
        out[0:rows.shape[0], 16 * k:16 * k + 16] = rows
    return out


def _prep(inputs):
    f = lambda k: np.asarray(inputs[k], np.float32)
    enc_names = ["last_agent_enc_out", "current_user_inform_enc_out",
                 "current_user_request_enc_out", "current_agent_propose_enc_out",
                 "current_agent_request_enc_out", "user_enc_out"]
    encs = [f(k) for k in enc_names]
    q = f("query_last_hidden")[0]
    kb = f("kb_turn")[0]
    lc = f("last_continue")[0]
    la = f("last_act")[0]
    ls = f("last_slot")[0]
    h0 = f("cas_last_hidden")[0]
    attn_W = f("attn_W"); attn_b = f("attn_b"); attn_v = f("attn_v")
    gWih = f("gru_Wih"); gWhh = f("gru_Whh")
    gbih = f("gru_bih"); gbhh = f("gru_bhh")

    # ---- shared weights ----
    W1T = np.zeros((3, 6, 1152, H), np.float32)
    W2T = np.zeros((3, 6, H, H), np.float32)
    vTa = np.zeros((3, 6, H), np.float32)
    for i in range(3):
        for j in range(6):
            W1T[i, j, 0:H] = attn_W[i, j, :, :H].T
            W1T[i, j, H] = attn_b[i, j]
            W2T[i, j] = attn_W[i, j, :, H:].T
            vTa[i, j] = attn_v[i, j]
    W1T = W1T.astype(bf16); W2T = W2T.astype(bf16); vTa = vTa.astype(bf16)

    # gin semantic order: [x(1024), ctx_0..5 (6*1024), kb(6), ones]
    # original col order stage0: [kb, x, ca0, ca1, ca2, ca4, ca3, ca5]
    # stages 1/2: [kb, x, a0..a5]
    def gin_perm(i):
        p = [np.arange(6, 1030)]
        if i == 0:
            blocks = [0, 1, 2, 4, 3, 5]  # ctx_j found at position blocks[j]
        else:
            blocks = [0, 1, 2, 3, 4, 5]
        for j in range(6):
            st = 1030 + 1024 * blocks[j]
            p.append(np.arange(st, st + 1024))
        p.append(np.arange(0, 6))  # kb cols
        return np.concatenate(p)

    WihT = np.zeros((3, GK * 128, 3 * H), np.float32)
    for i in range(3):
        perm = gin_perm(i)
        Wp = gWih[i][:, perm]       # [3072, 7174] -> semantic order
        WihT[i, 0:7174] = Wp.T
        WihT[i, 7174] = gbih[i]
    WihT = WihT.astype(bf16)

    WhhT = np.zeros((3, 1152, 3 * H), np.float32)
    for i in range(3):
        WhhT[i, 0:H] = gWhh[i].T
        WhhT[i, H] = gbhh[i]
    WhhT = WhhT.astype(bf16)

    # out proj: semantic order [h(1024), ctx_0..5, bias]
    ow = [f("cont_out_W"), f("act_out_W"), f("slot_out_W")]
    ob_ = [f("cont_out_b"), f("act_out_b"), f("slot_out_b")]
    WoT = np.zeros((3, GK * 128, 64), np.float32)
    for i in range(3):
        blocks = [0, 1, 2, 4, 3, 5] if i == 0 else [0, 1, 2, 3, 4, 5]
        Cd = ow[i].shape[0]
        WoT[i, 0:H, 0:Cd] = ow[i][:, 0:H].T
        for j in range(6):
            st = H + 1024 * blocks[j]
            WoT[i, H + 1024 * j:H + 1024 * (j + 1), 0:Cd] = ow[i][:, st:st + 1024].T
        WoT[i, 7168, 0:Cd] = ob_[i]
    WoT = WoT.astype(bf16)

    # input projections (semantic order happens to equal original order)
    WinT0 = np.zeros((9 * 128, H), np.float32)
    WinT0[0:1123] = f("cont_in_W").T
    WinT0[1123] = f("cont_in_b")
    WinT1 = np.zeros((2 * 128, H), np.float32)
    WinT1[0:99] = f("act_in_W").T
    WinT1[128] = f("act_in_b")
    sw = f("slot_in_W").T
    WinT2 = np.zeros((4 * 128, H), np.float32)
    WinT2[0:3] = sw[0:3]          # cont_out slot (chunk 0)
    WinT2[128:160] = sw[3:35]     # act_out slot (chunk 1)
    WinT2[256:288] = sw[35:67]    # la (chunk 2 rows 0:32)
    WinT2[288:352] = sw[67:131]   # ls (chunk 2 rows 32:96)
    WinT2[384] = f("slot_in_b")   # bias (chunk 3 row 0)
    WinT0 = WinT0.astype(bf16); WinT1 = WinT1.astype(bf16); WinT2 = WinT2.astype(bf16)

    ident_f = np.eye(128, dtype=np.float32)
    ident_b = np.eye(128, dtype=np.float32).astype(bf16)
    ones16 = np.zeros((128, 16), np.float32); ones16[0, :] = 1.0
    ones16 = ones16.astype(bf16)
    mask16 = np.zeros((128, 16), np.float32)
    for p in range(128):
        mask16[p, p % 16] = 1.0
    mask16 = mask16.astype(bf16)

    shared = dict(W1T=W1T, W2T=W2T, vT=vTa, WihT=WihT, WhhT=WhhT, WoT=WoT,
                  WinT0=WinT0, WinT1=WinT1, WinT2=WinT2,
                  ident_f=ident_f, ident_b=ident_b, ones16=ones16, mask16=mask16)

    in_maps = []
    for c in range(NC):
        bs, be = BL * c, BL * (c + 1)
        m = dict(shared)
        for j in range(6):
            sl = _to_bf16(encs[j][:, bs:be, :]).reshape(R, H)
            m[f"en{j}"] = sl
            m[f"eT{j}"] = np.ascontiguousarray(sl.T)
        c0 = np.zeros((1152, 16), np.float32)
        c0[0:H] = q[bs:be].T
        c0[H:H + 3] = lc[bs:be].T
        c0[H + 3:H + 35] = la[bs:be].T
        c0[H + 35:H + 99] = ls[bs:be].T
        c0[1123] = 1.0
        m["cat0T"] = _fold(c0.astype(bf16))
        b1 = np.zeros((128, 16), np.float32)
        b1[3:35] = la[bs:be].T
        b1[35:99] = ls[bs:be].T
        m["base1T_in"] = b1.astype(bf16)
        b2 = np.zeros((128, 48), np.float32)
        b2[0:32, 32:48] = la[bs:be].T
        b2[32:96, 32:48] = ls[bs:be].T
        m["base2T_in"] = b2.astype(bf16)
        kbt = np.zeros((128, 16), np.float32)
        kbt[0:6] = kb[bs:be].T
        kbt[6] = 1.0
        m["kbpadT"] = kbt.astype(bf16)
        m["h0T"] = _fold(np.ascontiguousarray(h0[bs:be].T, np.float32))
        in_maps.append(m)
    return in_maps


def _get_runner():
    """Build (once) a cached jax-jitted SPMD executor for the Bass module.

    Mirrors concourse.bass2jax.run_bass_via_pjrt's multi-core path but keeps
    the jitted callable so repeated invocations do not recompile.
    """
    if "runner" in _CACHE:
        return _CACHE["runner"]
    import jax
    import concourse.mybir as mybir
    from concourse import bass2jax
    from jax.sharding import Mesh, PartitionSpec
    from jax.experimental.shard_map import shard_map

    if "nc" not in _CACHE:
        _CACHE["nc"] = _build()
    nc = _CACHE["nc"]
    bass2jax.install_neuronx_cc_hook()

    part_name = nc.partition_id_tensor.name if nc.partition_id_tensor else None
    in_names, out_names, out_avals, zero_shapes = [], [], [], []
    for alloc in nc.m.functions[0].allocations:
        if not isinstance(alloc, mybir.MemoryLocationSet):
            continue
        name = alloc.memorylocations[0].name
        if alloc.kind == "ExternalInput":
            if name != part_name:
                in_names.append(name)
        elif alloc.kind == "ExternalOutput":
            out_names.append(name)
            shape = tuple(alloc.tensor_shape)
            dtype = mybir.dt.np(alloc.dtype)
            out_avals.append(jax.core.ShapedArray(shape, dtype))
            zero_shapes.append((shape, dtype))
    n_params = len(in_names)
    all_names = in_names + out_names
    if part_name is not None:
        all_names = all_names + [part_name]
    donate = tuple(range(n_params, n_params + len(out_names)))

    def _body(*args):
        operands = list(args)
        if part_name is not None:
            operands.append(bass2jax.partition_id_tensor())
        outs = bass2jax._bass_exec_p.bind(
            *operands,
            out_avals=tuple(out_avals),
            in_names=tuple(all_names),
            out_names=tuple(out_names),
            lowering_input_output_aliases=(),
            sim_require_finite=True,
            sim_require_nnan=True,
            nc=nc,
        )
        return tuple(outs)

    devices = jax.devices()[:NC]
    mesh = Mesh(np.asarray(devices), ("core",))
    nio = n_params + len(out_names)
    sharded = jax.jit(
        shard_map(_body, mesh=mesh,
                  in_specs=(PartitionSpec("core"),) * nio,
                  out_specs=(PartitionSpec("core"),) * len(out_names),
                  check_rep=False),
        donate_argnums=donate, keep_unused=True)

    def run(in_maps):
        concat_in = [
            np.concatenate([np.asarray(in_maps[c][n]) for c in range(NC)], axis=0)
            for n in in_names
        ]
        concat_zeros = [np.zeros((NC * s[0], *s[1:]), d) for s, d in zero_shapes]
        out_arrs = sharded(*concat_in, *concat_zeros)
        return [
            {n: np.asarray(out_arrs[i]).reshape(NC, *zero_shapes[i][0])[c]
             for i, n in enumerate(out_names)}
            for c in range(NC)
        ]

    runner = (run, sharded, in_names, zero_shapes, out_names)
    _CACHE["runner"] = runner
    return runner


def kernel(**inputs):
    run = _get_runner()[0]
    in_maps = _prep(inputs)
    results = run(in_maps)
    cont = np.zeros((1, B, C), np.float32)
    act = np.zeros((1, B, A_), np.float32)
    slot = np.zeros((1, B, S), np.float32)
    h = np.zeros((1, B, H), np.float32)
    for c in range(NC):
        bs, be = BL * c, BL * (c + 1)
        cont[0, bs:be] = results[c]["out_cont"]
        act[0, bs:be] = results[c]["out_act"]
        slot[0, bs:be] = results[c]["out_slot"]
        h[0, bs:be] = results[c]["out_h"]
    return cont, act, slot, h
